# revision 6
# baseline (speedup 1.0000x reference)
"""CausalWanSelfAttention on 8 trn2 NeuronCores.

Sharding: 8 cores = 4 head-groups (3 heads each) x 2 key-ranges.
Uniform SPMD program; per-core behavior comes only from input data:
  core c -> group g = c//2 (heads 3g..3g+2), range r = c%2.
  r=0 attends cache keys [0:5376); r=1 attends cache [5376:10560) (+192
  zero-pad) plus the 880 new keys. On r=0 cores the k-side rope tables and
  V weights are zeroed, so its "new key" scores are exactly 0 -> exp = 1,
  PV contribution 0; the constant pollution (880 + 192 = 1072 per query)
  is subtracted from the AllReduced rowsum on device.
Layouts: everything s-indexed uses 1024-col "strips" (real cols 0:440 and
512:952) so each fp32 matmul output (N=440) stays inside one PSUM bank.
Matmuls run in float32r (FP22 mantissa, full PE rate). Two tiny 8-core
AllReduces: (1) q/k rms sum-of-squares, (2) per-head softmax rowsums.
Host side: pre-transposes/permutes inputs, sums the 8 row-parallel output
partials.
"""

import sys
sys.path.insert(0, "/opt/trn_rl_repo")
import numpy as np

import concourse.bacc as bacc
import concourse.mybir as mybir
from concourse import tile
from concourse.bass_utils import run_bass_kernel_spmd

f32 = mybir.dt.float32
f32r = mybir.dt.float32r
AF = mybir.ActivationFunctionType
MUL = mybir.AluOpType.mult
ADD = mybir.AluOpType.add

DIM, NH, HD, S = 1536, 12, 128, 880
CACHE_USED = 10560            # keys already in cache (current_start)
KC = 5376                     # cache keys per core (42 tiles of 128)
NKT_C = KC // 128             # 42 cache k-tiles
NCORES, HG = 8, 3             # cores, heads per group
EPS = 1e-6
H0, H1 = 440, 512             # strip: half width, second-half offset
RS_CORR = 1072.0              # 880 (r0 zeroed-new) + 192 (r1 pad) exp(0)=1
NEW_TILES = [128, 128, 128, 128, 128, 128, 112]   # 880 new keys
HALVES = ((0, 0), (H1, 440))  # (strip col offset, compact col offset)

_prog = None
last_results = None


def _build():
    nc = bacc.Bacc(None, target_bir_lowering=False, debug=False,
                   num_devices=NCORES)

    d_xT = nc.dram_tensor("xT", [12, 128, S], f32r, kind="ExternalInput")
    d_wq = nc.dram_tensor("wq", [12, 128, 384], f32r, kind="ExternalInput")
    d_wk = nc.dram_tensor("wk", [12, 128, 384], f32r, kind="ExternalInput")
    d_wv = nc.dram_tensor("wv", [12, 128, 384], f32r, kind="ExternalInput")
    d_wo = nc.dram_tensor("wo", [3, 128, DIM], f32r, kind="ExternalInput")
    d_bias = nc.dram_tensor("bias", [1, 2688], f32r, kind="ExternalInput")
    d_cosq = nc.dram_tensor("cosq", [3, 128, 1024], f32, kind="ExternalInput")
    d_sinq = nc.dram_tensor("sinq", [128, 1024], f32, kind="ExternalInput")
    d_cosk = nc.dram_tensor("cosk", [3, 128, 1024], f32, kind="ExternalInput")
    d_sink = nc.dram_tensor("sink", [128, 1024], f32, kind="ExternalInput")
    d_swmq = nc.dram_tensor("swmq", [3, 128, 128], f32r, kind="ExternalInput")
    d_swmk = nc.dram_tensor("swmk", [3, 128, 128], f32r, kind="ExternalInput")
    d_kcT = nc.dram_tensor("kcT", [3, 128, KC], f32r, kind="ExternalInput")
    d_vc = nc.dram_tensor("vc", [3, KC, 128], f32r, kind="ExternalInput")
    d_ones = nc.dram_tensor("ones", [128, 8], f32r, kind="ExternalInput")
    d_oneA = nc.dram_tensor("oneA", [1, 36], f32r, kind="ExternalInput")
    d_oneB = nc.dram_tensor("oneB", [12, 3], f32r, kind="ExternalInput")
    d_cvec = nc.dram_tensor("cvec", [1, 8], f32, kind="ExternalInput")
    d_out = nc.dram_tensor("yT", [12, 128, 1024], f32, kind="ExternalOutput")

    rg = [list(range(NCORES))]

    with tile.TileContext(nc) as tc:
        with (
            tc.tile_pool(name="pres", bufs=1) as pres,
            tc.tile_pool(name="pdram", bufs=1, space="DRAM") as pdram,
        ):
            # ---- small resident loads ---------------------------------
            bias_s = pres.tile([1, 2688], f32r, tag="bias")
            nc.sync.dma_start(bias_s[:], d_bias.ap())
            ones_s = pres.tile([128, 8], f32r, tag="ones")
            nc.sync.dma_start(ones_s[:], d_ones.ap())
            oneA_s = pres.tile([1, 36], f32r, tag="oneA")
            nc.sync.dma_start(oneA_s[:], d_oneA.ap())
            oneB_s = pres.tile([12, 3], f32r, tag="oneB")
            nc.sync.dma_start(oneB_s[:], d_oneB.ap())
            cvec_s = pres.tile([1, 8], f32, tag="cvec")
            nc.sync.dma_start(cvec_s[:], d_cvec.ap())
            swmq_s, swmk_s = [], []
            for h in range(HG):
                t = pres.tile([128, 128], f32r, tag=f"swmq{h}")
                nc.sync.dma_start(t[:], d_swmq.ap()[h])
                swmq_s.append(t)
                t = pres.tile([128, 128], f32r, tag=f"swmk{h}")
                nc.sync.dma_start(t[:], d_swmk.ap()[h])
                swmk_s.append(t)
            ones_row = ones_s[0:1, 0:1]

            v_sb = [pres.tile([128, 384], f32r, tag=f"vsb{st}",
                              name=f"vsb{st}") for st in range(7)]
            qro = [pres.tile([128, 1024], f32r, tag=f"qro{h}",
                             name=f"qro{h}") for h in range(HG)]
            kro = [pres.tile([128, S], f32r, tag=f"kro{h}",
                             name=f"kro{h}") for h in range(HG)]

            with tc.tile_pool(name="pqk", bufs=1) as pqk:
                # ---- phase 1: q/k/v projections -----------------------
                with (
                    tc.tile_pool(name="p1io", bufs=1) as p1io,
                    tc.tile_pool(name="ppj", bufs=2, space="PSUM") as ppj,
                ):
                    xT = p1io.tile([128, 12 * S], f32r, tag="xT")
                    nc.sync.dma_start(
                        xT[:].rearrange("p (t e) -> p t e", t=12),
                        d_xT.ap().rearrange("t p e -> p t e"))
                    w_in = {}
                    for nm, dt_ in (("wq", d_wq), ("wk", d_wk), ("wv", d_wv)):
                        w = p1io.tile([128, 12 * 384], f32r, tag=nm)
                        nc.sync.dma_start(
                            w[:].rearrange("p (t e) -> p t e", t=12),
                            dt_.ap().rearrange("t p e -> p t e"))
                        w_in[nm] = w

                    def qk_proj(w_s, bias_off, out_tag):
                        outs = []
                        for h in range(HG):
                            ps = ppj.tile([128, 1024], f32, tag="proj")
                            for po, so in HALVES:
                                for t in range(12):
                                    nc.tensor.matmul(
                                        ps[:, po:po + H0],
                                        w_s[:, t * 384 + 128 * h:
                                            t * 384 + 128 * (h + 1)],
                                        xT[:, t * S + so:t * S + so + H0],
                                        start=(t == 0), stop=False)
                                nc.tensor.matmul(
                                    ps[:, po:po + H0],
                                    bias_s[0:1, bias_off + 128 * h:
                                           bias_off + 128 * (h + 1)],
                                    ones_row.broadcast_to([1, H0]),
                                    start=False, stop=True)
                            sb = pqk.tile([128, 1024], f32r,
                                          tag=f"{out_tag}{h}")
                            nc.scalar.copy(sb[:], ps[:])
                            outs.append(sb)
                        return outs

                    q_sb = qk_proj(w_in["wq"], 0, "qsb")
                    k_sb = qk_proj(w_in["wk"], 384, "ksb")

                    for st in range(7):
                        s0 = st * 128
                        sz = min(128, S - s0)
                        ps = ppj.tile([128, 384], f32, tag="vproj")
                        for t in range(12):
                            nc.tensor.matmul(
                                ps[0:sz, :],
                                xT[:, t * S + s0:t * S + s0 + sz],
                                w_in["wv"][:, t * 384:(t + 1) * 384],
                                start=(t == 0), stop=False)
                        nc.tensor.matmul(ps[0:sz, :],
                                         ones_row.broadcast_to([1, sz]),
                                         bias_s[0:1, 768:1152],
                                         start=False, stop=True)
                        nc.scalar.copy(v_sb[st][0:sz, :], ps[0:sz, :])

                # ---- phases 2-3: rms sumsq, AR#1, rope ----------------
                with (
                    tc.tile_pool(name="p23", bufs=1) as p23,
                    tc.tile_pool(name="p23v", bufs=4) as p23v,
                    tc.tile_pool(name="p23w", bufs=2) as p23w,
                    tc.tile_pool(name="p23ps", bufs=1, space="PSUM") as p23ps,
                    tc.tile_pool(name="pswp", bufs=2, space="PSUM") as pswp,
                ):
                    ssq = p23ps.tile([1, 1024], f32, tag="ssq")
                    ssk = p23ps.tile([1, 1024], f32, tag="ssk")
                    for acc, src in ((ssq, q_sb), (ssk, k_sb)):
                        for h in range(HG):
                            for po, _ in HALVES:
                                sq = p23w.tile([128, H0], f32r, tag="sqt")
                                nc.scalar.activation(
                                    sq[:], src[h][:, po:po + H0], AF.Square)
                                nc.tensor.matmul(
                                    acc[0:1, po:po + H0],
                                    ones_s[:, 0:1], sq[:],
                                    start=(h == 0), stop=(h == HG - 1))
                    stq = p23v.tile([1, 1024], f32, tag="vec")
                    nc.vector.tensor_copy(stq[:], ssq[:])
                    stk = p23v.tile([1, 1024], f32, tag="vec")
                    nc.vector.tensor_copy(stk[:], ssk[:])
                    b1i = pdram.tile([2, 1024], f32)
                    b1o = pdram.tile([2, 1024], f32)
                    nc.sync.dma_start(b1i[0:1, :], stq[:])
                    nc.sync.dma_start(b1i[1:2, :], stk[:])
                    nc.gpsimd.collective_compute(
                        "AllReduce", ADD, replica_groups=rg,
                        ins=[b1i[:].opt()], outs=[b1o[:].opt()])
                    arq = p23v.tile([1, 1024], f32, tag="vec")
                    nc.sync.dma_start(arq[:], b1o[0:1, :])
                    ark = p23v.tile([1, 1024], f32, tag="vec")
                    nc.sync.dma_start(ark[:], b1o[1:2, :])
                    # rms_q also folds the 1/sqrt(HD) score scale:
                    # 1/sqrt(128*(mean+eps))
                    sqq = p23v.tile([1, 1024], f32, tag="vec")
                    nc.scalar.activation(sqq[:], arq[:], AF.Sqrt,
                                         scale=128.0 / (2 * DIM),
                                         bias=cvec_s[0:1, 0:1])
                    sqk = p23v.tile([1, 1024], f32, tag="vec")
                    nc.scalar.activation(sqk[:], ark[:], AF.Sqrt,
                                         scale=1.0 / (2 * DIM),
                                         bias=cvec_s[0:1, 1:2])
                    rcq = p23v.tile([1, 1024], f32, tag="vec")
                    nc.vector.reciprocal(rcq[:], sqq[:])
                    rck = p23v.tile([1, 1024], f32, tag="vec")
                    nc.vector.reciprocal(rck[:], sqk[:])
                    rbq = p23.tile([128, 1024], f32, tag="rbq")
                    nc.gpsimd.partition_broadcast(rbq[:], rcq[:])
                    rbk = p23.tile([128, 1024], f32, tag="rbk")
                    nc.gpsimd.partition_broadcast(rbk[:], rck[:])

                    cosq_s, cosk_s = [], []
                    for h in range(HG):
                        t = p23.tile([128, 1024], f32, tag=f"cosq{h}")
                        nc.sync.dma_start(t[:], d_cosq.ap()[h])
                        cosq_s.append(t)
                        t = p23.tile([128, 1024], f32, tag=f"cosk{h}")
                        nc.sync.dma_start(t[:], d_cosk.ap()[h])
                        cosk_s.append(t)
                    sinq_s = p23.tile([128, 1024], f32, tag="sinq")
                    nc.sync.dma_start(sinq_s[:], d_sinq.ap())
                    sink_s = p23.tile([128, 1024], f32, tag="sink")
                    nc.sync.dma_start(sink_s[:], d_sink.ap())

                    for h in range(HG):
                        tmp = pswp.tile([128, 1024], f32, tag="swp")
                        for po, _ in HALVES:
                            nc.tensor.matmul(tmp[:, po:po + H0], swmq_s[h][:],
                                             q_sb[h][:, po:po + H0],
                                             start=True, stop=True)
                        u = p23w.tile([128, 1024], f32, tag="ropeu")
                        nc.vector.tensor_tensor(u[:], q_sb[h][:],
                                                cosq_s[h][:], op=MUL)
                        v = p23w.tile([128, 1024], f32, tag="ropev")
                        nc.vector.tensor_tensor(v[:], tmp[:], sinq_s[:],
                                                op=MUL)
                        w = p23w.tile([128, 1024], f32, tag="ropew")
                        nc.vector.tensor_tensor(w[:], u[:], v[:], op=ADD)
                        nc.vector.tensor_tensor(qro[h][:], w[:], rbq[:],
                                                op=MUL)
                    for h in range(HG):
                        tmp = pswp.tile([128, 1024], f32, tag="swp")
                        for po, _ in HALVES:
                            nc.tensor.matmul(tmp[:, po:po + H0], swmk_s[h][:],
                                             k_sb[h][:, po:po + H0],
                                             start=True, stop=True)
                        for po, so in HALVES:
                            a = p23w.tile([128, H0], f32, tag="ropea")
                            nc.vector.tensor_tensor(
                                a[:], k_sb[h][:, po:po + H0],
                                cosk_s[h][:, po:po + H0], op=MUL)
                            b = p23w.tile([128, H0], f32, tag="ropeb")
                            nc.vector.tensor_tensor(
                                b[:], tmp[:, po:po + H0],
                                sink_s[:, po:po + H0], op=MUL)
                            cc = p23w.tile([128, H0], f32, tag="ropec")
                            nc.vector.tensor_tensor(cc[:], a[:], b[:], op=ADD)
                            nc.vector.tensor_tensor(
                                kro[h][:, so:so + H0], cc[:],
                                rbk[:, po:po + H0], op=MUL)

            # ---- phase 4: attention -----------------------------------
            with (
                tc.tile_pool(name="p4", bufs=1) as p4,
                tc.tile_pool(name="p4s", bufs=2) as p4s,
                tc.tile_pool(name="p4p", bufs=3) as p4p,
                tc.tile_pool(name="p4v", bufs=4) as p4v,
                tc.tile_pool(name="p4r", bufs=3) as p4r,
            ):
                wo_s = p4.tile([128, 3 * DIM], f32r, tag="wo")
                nc.sync.dma_start(
                    wo_s[:].rearrange("p (t e) -> p t e", t=3),
                    d_wo.ap().rearrange("t p e -> p t e"))

                attn_psum = tc.tile_pool(name="psc", bufs=2, space="PSUM")
                psc = attn_psum.__enter__()
                acc_psum = tc.tile_pool(name="pacc", bufs=1, space="PSUM")
                pacc = acc_psum.__enter__()

                def attn_tile(oacc, rsacc, lk, lv, qro_h, m, first, last):
                    sc = psc.tile([128, 1024], f32, tag="sc")
                    for po, _ in HALVES:
                        nc.tensor.matmul(sc[0:m, po:po + H0], lk,
                                         qro_h[:, po:po + H0],
                                         start=True, stop=True)
                    pt = p4p.tile([128, 1024], f32r, tag="pt")
                    nc.scalar.activation(pt[0:m, :], sc[0:m, :], AF.Exp)
                    for po, _ in HALVES:
                        nc.tensor.matmul(oacc[:, po:po + H0], lv,
                                         pt[0:m, po:po + H0],
                                         start=first, stop=last)
                        nc.tensor.matmul(rsacc[0:1, po:po + H0],
                                         ones_s[0:m, 0:1],
                                         pt[0:m, po:po + H0],
                                         start=first, stop=last)

                o_un, rs_l = [], []
                n_kt = NKT_C + len(NEW_TILES)          # 49 k-tiles per head
                chunks = [(0, 2048), (2048, 2048), (4096, 1280)]
                for h in range(HG):
                    oacc = pacc.tile([128, 1024], f32, tag="oacc")
                    rsacc = pacc.tile([1, 1024], f32, tag="rsacc")
                    kt = 0
                    for c0, cn in chunks:
                        nt = cn // 128
                        kc = p4s.tile([128, 2048], f32r, tag="kc")
                        nc.sync.dma_start(kc[:, 0:cn],
                                          d_kcT.ap()[h, :, c0:c0 + cn])
                        vc = p4s.tile([128, 2048], f32r, tag="vc")
                        nc.sync.dma_start(
                            vc[:, 0:cn].rearrange("p (t e) -> p t e", t=nt),
                            d_vc.ap()[h, c0:c0 + cn, :].rearrange(
                                "(t p) e -> p t e", p=128))
                        for j in range(nt):
                            attn_tile(oacc, rsacc,
                                      kc[:, j * 128:(j + 1) * 128],
                                      vc[:, j * 128:(j + 1) * 128],
                                      qro[h], 128, kt == 0, kt == n_kt - 1)
                            kt += 1
                    for j, sz in enumerate(NEW_TILES):
                        attn_tile(oacc, rsacc,
                                  kro[h][:, j * 128:j * 128 + sz],
                                  v_sb[j][0:sz, 128 * h:128 * (h + 1)],
                                  qro[h], sz, kt == 0, kt == n_kt - 1)
                        kt += 1
                    ou = p4.tile([128, 1024], f32, tag=f"oun{h}")
                    nc.vector.tensor_copy(ou[:], oacc[:])
                    o_un.append(ou)
                    rl = p4r.tile([1, 1024], f32r, tag="rsl")
                    nc.vector.tensor_copy(rl[:], rsacc[:])
                    rs_l.append(rl)

                acc_psum.__exit__(None, None, None)
                attn_psum.__exit__(None, None, None)

                # ---- AllReduce #2: rowsums ----------------------------
                with tc.tile_pool(name="p45ps", bufs=1,
                                  space="PSUM") as p45ps:
                    b12 = p45ps.tile([12, 1024], f32, tag="b12")
                    for po, _ in HALVES:
                        for h in range(HG):
                            nc.tensor.matmul(
                                b12[:, po:po + H0],
                                oneA_s[0:1, 12 * h:12 * (h + 1)],
                                rs_l[h][0:1, po:po + H0],
                                start=(h == 0), stop=(h == HG - 1))
                    st12 = p4.tile([12, 1024], f32, tag="st12")
                    nc.vector.tensor_copy(st12[:], b12[:])
                    b2i = pdram.tile([12, 1024], f32)
                    b2o = pdram.tile([12, 1024], f32)
                    nc.sync.dma_start(b2i[:], st12[:])
                    nc.gpsimd.collective_compute(
                        "AllReduce", ADD, replica_groups=rg,
                        ins=[b2i[:].opt()], outs=[b2o[:].opt()])
                    ar2 = p4.tile([12, 1024], f32, tag="ar2")
                    nc.sync.dma_start(ar2[:], b2o[:])
                    ar2r = p4.tile([12, 1024], f32r, tag="ar2r")
                    nc.scalar.copy(ar2r[:], ar2[:])

                    od = []
                    for h in range(HG):
                        rst = p45ps.tile([1, 1024], f32, tag=f"rst{h}")
                        for po, _ in HALVES:
                            nc.tensor.matmul(rst[0:1, po:po + H0],
                                             oneB_s[:, h:h + 1],
                                             ar2r[:, po:po + H0],
                                             start=True, stop=True)
                        sub = p4v.tile([1, 1024], f32, tag="vec")
                        nc.vector.tensor_scalar_add(sub[:], rst[:], -RS_CORR)
                        rcp = p4v.tile([1, 1024], f32, tag="vec")
                        nc.vector.reciprocal(rcp[:], sub[:])
                        rb = p4s.tile([128, 1024], f32, tag="rb")
                        nc.gpsimd.partition_broadcast(rb[:], rcp[:])
                        o = p4.tile([128, 1024], f32r, tag=f"od{h}")
                        nc.vector.tensor_tensor(o[:], o_un[h][:], rb[:],
                                                op=MUL)
                        od.append(o)

                # ---- phase 5: output projection -----------------------
                with tc.tile_pool(name="p5ps", bufs=2, space="PSUM") as p5ps:
                    for d in range(12):
                        yp = p5ps.tile([128, 1024], f32, tag="yp")
                        for po, _ in HALVES:
                            for h in range(HG):
                                nc.tensor.matmul(
                                    yp[:, po:po + H0],
                                    wo_s[:, h * DIM + 128 * d:
                                         h * DIM + 128 * (d + 1)],
                                    od[h][:, po:po + H0],
                                    start=(h == 0), stop=False)
                            nc.tensor.matmul(
                                yp[:, po:po + H0],
                                bias_s[0:1, 1152 + 128 * d:
                                       1152 + 128 * (d + 1)],
                                ones_row.broadcast_to([1, H0]),
                                start=False, stop=True)
                        ysb = p4s.tile([128, 1024], f32, tag="ysb")
                        nc.vector.tensor_copy(ysb[:], yp[:])
                        nc.sync.dma_start(d_out.ap()[d], ysb[:])

    nc.compile()
    return nc


def _rope_tables(freqs, grid, start_frame):
    f, h, w = int(grid[0]), int(grid[1]), int(grid[2])
    c = HD // 2
    s0, s1 = c - 2 * (c // 3), c // 3
    out = []
    for tab in (freqs[..., 0], freqs[..., 1]):
        tf = np.broadcast_to(tab[start_frame:start_frame + f, :s0]
                             [:, None, None, :], (f, h, w, s0))
        th = np.broadcast_to(tab[:h, s0:s0 + s1][None, :, None, :],
                             (f, h, w, s1))
        tw = np.broadcast_to(tab[:w, s0 + s1:c][None, None, :, :],
                             (f, h, w, s1))
        out.append(np.concatenate([tf, th, tw], -1).reshape(f * h * w, c))
    return out  # pc, ps each [S, 64]


def _strip(a):
    # [..., 880] -> [..., 1024] with halves at 0 and 512
    out = np.zeros(a.shape[:-1] + (1024,), np.float32)
    out[..., 0:440] = a[..., 0:440]
    out[..., 512:952] = a[..., 440:880]
    return out


def _host_prep(inputs):
    x = np.asarray(inputs["x"], np.float32)
    wq, wk, wv, wo = (np.asarray(inputs[k], np.float32)
                      for k in ("wq", "wk", "wv", "wo"))
    bq, bk, bv, bo = (np.asarray(inputs[k], np.float32)
                      for k in ("bq", "bk", "bv", "bo"))
    gq = np.asarray(inputs["gq"], np.float32)
    gk = np.asarray(inputs["gk"], np.float32)
    freqs = np.asarray(inputs["freqs"], np.float32)
    k_cache = np.asarray(inputs["k_cache"], np.float32)
    v_cache = np.asarray(inputs["v_cache"], np.float32)
    grid = np.asarray(inputs["grid_sizes"]).reshape(-1)
    cur = int(inputs["current_start"])
    assert x.shape == (1, S, DIM) and cur == CACHE_USED, (x.shape, cur)
    assert int(grid[0]) * int(grid[1]) * int(grid[2]) == S

    start_frame = cur // (int(grid[1]) * int(grid[2]))
    pc_t, ps_t = _rope_tables(freqs, grid, start_frame)   # [S, 64]

    perm = np.concatenate([np.arange(0, HD, 2), np.arange(1, HD, 2)])
    cidx = np.arange(HD) % 64
    xT_a = np.ascontiguousarray(x[0].T).reshape(12, 128, S)

    cos_b = np.ascontiguousarray(pc_t[:, cidx].T)         # [128, S]
    sin_b = np.ascontiguousarray(ps_t[:, cidx].T)

    in_maps = []
    for c in range(NCORES):
        g, r = c // 2, c % 2
        heads = [3 * g + h for h in range(HG)]
        row_idx = np.concatenate([128 * gh + perm for gh in heads])
        vrow_idx = np.concatenate([128 * gh + np.arange(HD) for gh in heads])

        wq_a = np.ascontiguousarray(wq[row_idx].T).reshape(12, 128, 384)
        wk_a = np.ascontiguousarray(wk[row_idx].T).reshape(12, 128, 384)
        if r == 1:
            wv_a = np.ascontiguousarray(wv[vrow_idx].T).reshape(12, 128, 384)
            bv_l = bv[vrow_idx]
        else:
            wv_a = np.zeros((12, 128, 384), np.float32)
            bv_l = np.zeros(384, np.float32)
        wo_a = np.ascontiguousarray(wo[:, vrow_idx].T).reshape(3, 128, DIM)
        bias_a = np.zeros((1, 2688), np.float32)
        bias_a[0, 0:384] = bq[row_idx]
        bias_a[0, 384:768] = bk[row_idx]
        bias_a[0, 768:1152] = bv_l
        if c == 0:
            bias_a[0, 1152:2688] = bo

        def side_tables(gvec, zero):
            cos3 = np.zeros((3, 128, 1024), np.float32)
            swm3 = np.zeros((3, 128, 128), np.float32)
            if zero:
                return cos3, np.zeros((128, 1024), np.float32), swm3
            for h, gh in enumerate(heads):
                gp = gvec[128 * gh + perm]                    # [128]
                cos3[h] = _strip(gp[:, None] * cos_b)
                for i in range(64):
                    swm3[h, 64 + i, i] = -gp[64 + i]
                    swm3[h, i, 64 + i] = gp[i]
            return cos3, _strip(sin_b), swm3

        cosq_a, sinq_a, swmq_a = side_tables(gq, False)
        cosk_a, sink_a, swmk_a = side_tables(gk, r == 0)

        kcT_a = np.zeros((3, 128, KC), np.float32)
        vc_a = np.zeros((3, KC, 128), np.float32)
        lo, hi = (0, KC) if r == 0 else (KC, CACHE_USED)
        n = hi - lo
        for h, gh in enumerate(heads):
            kcT_a[h, :, 0:n] = k_cache[0, lo:hi, gh, :][:, perm].T
            vc_a[h, 0:n, :] = v_cache[0, lo:hi, gh, :]

        oneA = np.zeros((1, 36), np.float32)
        oneB = np.zeros((12, 3), np.float32)
        for h, gh in enumerate(heads):
            oneA[0, 12 * h + gh] = 1.0
            oneB[gh, h] = 1.0

        in_maps.append(dict(
            xT=xT_a, wq=wq_a, wk=wk_a, wv=wv_a, wo=wo_a, bias=bias_a,
            cosq=cosq_a, sinq=sinq_a, cosk=cosk_a, sink=sink_a,
            swmq=swmq_a, swmk=swmk_a, kcT=np.ascontiguousarray(kcT_a),
            vc=vc_a, ones=np.ones((128, 8), np.float32),
            oneA=oneA, oneB=oneB,
            cvec=np.array([[128.0 * EPS, EPS, 0, 0, 0, 0, 0, 0]],
                          np.float32)))
    return in_maps


def kernel(**inputs):
    global _prog, last_results
    if _prog is None:
        _prog = _build()
    in_maps = _host_prep(inputs)
    r = run_bass_kernel_spmd(_prog, in_maps, core_ids=list(range(NCORES)))
    last_results = r
    acc = np.zeros((12, 128, 1024), np.float64)
    for c in range(NCORES):
        acc += r.results[c]["yT"]
    yT = np.concatenate([acc[:, :, 0:440], acc[:, :, 512:952]], axis=2)
    y = yT.reshape(DIM, S).T
    return np.ascontiguousarray(y).reshape(1, S, DIM).astype(np.float32)


# revision 8
# speedup vs baseline: 1.0009x; 1.0009x over previous
"""CausalWanSelfAttention on 8 trn2 NeuronCores.

Sharding: 8 cores = 4 head-groups (3 heads each) x 2 key-ranges.
Uniform SPMD program; per-core behavior comes only from input data:
  core c -> group g = c//2 (heads 3g..3g+2), range r = c%2.
  r=0 attends cache keys [0:5376); r=1 attends cache [5376:10560) (+192
  zero-pad) plus the 880 new keys. On r=0 cores the k-side rope tables and
  V weights are zeroed, so its "new key" scores are exactly 0 -> exp = 1,
  PV contribution 0; the constant pollution (880 + 192 = 1072 per query)
  is subtracted from the AllReduced rowsum on device.
Layouts: everything s-indexed uses 1024-col "strips" (real cols 0:440 and
512:952) so each fp32 matmul output (N=440) stays inside one PSUM bank.
Matmuls run in float32r (FP22 mantissa, full PE rate). Two tiny 8-core
AllReduces: (1) q/k rms sum-of-squares, (2) per-head softmax rowsums.
Host side: pre-transposes/permutes inputs, sums the 8 row-parallel output
partials.
"""

import sys
sys.path.insert(0, "/opt/trn_rl_repo")
import numpy as np

import concourse.bacc as bacc
import concourse.mybir as mybir
from concourse import tile
from concourse.bass_utils import run_bass_kernel_spmd

f32 = mybir.dt.float32
f32r = mybir.dt.float32r
AF = mybir.ActivationFunctionType
MUL = mybir.AluOpType.mult
ADD = mybir.AluOpType.add

DIM, NH, HD, S = 1536, 12, 128, 880
CACHE_USED = 10560            # keys already in cache (current_start)
KC = 5376                     # cache keys per core (42 tiles of 128)
NKT_C = KC // 128             # 42 cache k-tiles
NCORES, HG = 8, 3             # cores, heads per group
EPS = 1e-6
H0, H1 = 440, 512             # strip: half width, second-half offset
RS_CORR = 1072.0              # 880 (r0 zeroed-new) + 192 (r1 pad) exp(0)=1
NEW_TILES = [128, 128, 128, 128, 128, 128, 112]   # 880 new keys
HALVES = ((0, 0), (H1, 440))  # (strip col offset, compact col offset)

_prog = None
last_results = None


def _build():
    nc = bacc.Bacc(None, target_bir_lowering=False, debug=False,
                   num_devices=NCORES)

    d_xT = nc.dram_tensor("xT", [128, 12 * S], f32r, kind="ExternalInput")
    d_wq = nc.dram_tensor("wq", [128, 12 * 384], f32r, kind="ExternalInput")
    d_wk = nc.dram_tensor("wk", [128, 12 * 384], f32r, kind="ExternalInput")
    d_wv = nc.dram_tensor("wv", [128, 12 * 384], f32r, kind="ExternalInput")
    d_wo = nc.dram_tensor("wo", [128, 3 * DIM], f32r, kind="ExternalInput")
    d_bias = nc.dram_tensor("bias", [1, 2688], f32r, kind="ExternalInput")
    d_cosq = nc.dram_tensor("cosq", [3, 128, 1024], f32, kind="ExternalInput")
    d_sinq = nc.dram_tensor("sinq", [128, 1024], f32, kind="ExternalInput")
    d_cosk = nc.dram_tensor("cosk", [3, 128, 1024], f32, kind="ExternalInput")
    d_sink = nc.dram_tensor("sink", [128, 1024], f32, kind="ExternalInput")
    d_swmq = nc.dram_tensor("swmq", [3, 128, 128], f32r, kind="ExternalInput")
    d_swmk = nc.dram_tensor("swmk", [3, 128, 128], f32r, kind="ExternalInput")
    d_kcT = nc.dram_tensor("kcT", [3, 128, KC], f32r, kind="ExternalInput")
    d_vc = nc.dram_tensor("vc", [3, 3, 128, 2048], f32r, kind="ExternalInput")
    d_ones = nc.dram_tensor("ones", [128, 8], f32r, kind="ExternalInput")
    d_oneA = nc.dram_tensor("oneA", [1, 36], f32r, kind="ExternalInput")
    d_oneB = nc.dram_tensor("oneB", [12, 3], f32r, kind="ExternalInput")
    d_cvec = nc.dram_tensor("cvec", [1, 8], f32, kind="ExternalInput")
    d_out = nc.dram_tensor("yT", [12, 128, 1024], f32, kind="ExternalOutput")

    rg = [list(range(NCORES))]

    with tile.TileContext(nc) as tc:
        with (
            tc.tile_pool(name="pres", bufs=1) as pres,
            tc.tile_pool(name="pdram", bufs=1, space="DRAM") as pdram,
        ):
            # ---- small resident loads ---------------------------------
            bias_s = pres.tile([1, 2688], f32r, tag="bias")
            nc.sync.dma_start(bias_s[:], d_bias.ap())
            ones_s = pres.tile([128, 8], f32r, tag="ones")
            nc.sync.dma_start(ones_s[:], d_ones.ap())
            oneA_s = pres.tile([1, 36], f32r, tag="oneA")
            nc.sync.dma_start(oneA_s[:], d_oneA.ap())
            oneB_s = pres.tile([12, 3], f32r, tag="oneB")
            nc.sync.dma_start(oneB_s[:], d_oneB.ap())
            cvec_s = pres.tile([1, 8], f32, tag="cvec")
            nc.sync.dma_start(cvec_s[:], d_cvec.ap())
            swmq_s, swmk_s = [], []
            for h in range(HG):
                t = pres.tile([128, 128], f32r, tag=f"swmq{h}")
                nc.sync.dma_start(t[:], d_swmq.ap()[h])
                swmq_s.append(t)
                t = pres.tile([128, 128], f32r, tag=f"swmk{h}")
                nc.sync.dma_start(t[:], d_swmk.ap()[h])
                swmk_s.append(t)
            ones_row = ones_s[0:1, 0:1]

            v_sb = [pres.tile([128, 384], f32r, tag=f"vsb{st}",
                              name=f"vsb{st}") for st in range(7)]
            qro = [pres.tile([128, 1024], f32r, tag=f"qro{h}",
                             name=f"qro{h}") for h in range(HG)]
            kro = [pres.tile([128, S], f32r, tag=f"kro{h}",
                             name=f"kro{h}") for h in range(HG)]

            with tc.tile_pool(name="pqk", bufs=1) as pqk:
                # ---- phase 1: q/k/v projections -----------------------
                with (
                    tc.tile_pool(name="p1io", bufs=1) as p1io,
                    tc.tile_pool(name="ppj", bufs=2, space="PSUM") as ppj,
                ):
                    xT = p1io.tile([128, 12 * S], f32r, tag="xT")
                    nc.sync.dma_start(xT[:], d_xT.ap())
                    w_in = {}
                    for nm, dt_ in (("wq", d_wq), ("wk", d_wk), ("wv", d_wv)):
                        w = p1io.tile([128, 12 * 384], f32r, tag=nm)
                        nc.sync.dma_start(w[:], dt_.ap())
                        w_in[nm] = w

                    def qk_proj(w_s, bias_off, out_tag):
                        outs = []
                        for h in range(HG):
                            ps = ppj.tile([128, 1024], f32, tag="proj")
                            for po, so in HALVES:
                                for t in range(12):
                                    nc.tensor.matmul(
                                        ps[:, po:po + H0],
                                        w_s[:, t * 384 + 128 * h:
                                            t * 384 + 128 * (h + 1)],
                                        xT[:, t * S + so:t * S + so + H0],
                                        start=(t == 0), stop=False)
                                nc.tensor.matmul(
                                    ps[:, po:po + H0],
                                    bias_s[0:1, bias_off + 128 * h:
                                           bias_off + 128 * (h + 1)],
                                    ones_row.broadcast_to([1, H0]),
                                    start=False, stop=True)
                            sb = pqk.tile([128, 1024], f32r,
                                          tag=f"{out_tag}{h}")
                            nc.scalar.copy(sb[:], ps[:])
                            outs.append(sb)
                        return outs

                    q_sb = qk_proj(w_in["wq"], 0, "qsb")
                    k_sb = qk_proj(w_in["wk"], 384, "ksb")

                    for st in range(7):
                        s0 = st * 128
                        sz = min(128, S - s0)
                        ps = ppj.tile([128, 384], f32, tag="vproj")
                        for t in range(12):
                            nc.tensor.matmul(
                                ps[0:sz, :],
                                xT[:, t * S + s0:t * S + s0 + sz],
                                w_in["wv"][:, t * 384:(t + 1) * 384],
                                start=(t == 0), stop=False)
                        nc.tensor.matmul(ps[0:sz, :],
                                         ones_row.broadcast_to([1, sz]),
                                         bias_s[0:1, 768:1152],
                                         start=False, stop=True)
                        nc.scalar.copy(v_sb[st][0:sz, :], ps[0:sz, :])

                # ---- phases 2-3: rms sumsq, AR#1, rope ----------------
                with (
                    tc.tile_pool(name="p23", bufs=1) as p23,
                    tc.tile_pool(name="p23v", bufs=4) as p23v,
                    tc.tile_pool(name="p23w", bufs=2) as p23w,
                    tc.tile_pool(name="p23ps", bufs=1, space="PSUM") as p23ps,
                    tc.tile_pool(name="pswp", bufs=2, space="PSUM") as pswp,
                ):
                    ssq = p23ps.tile([1, 1024], f32, tag="ssq")
                    ssk = p23ps.tile([1, 1024], f32, tag="ssk")
                    for acc, src in ((ssq, q_sb), (ssk, k_sb)):
                        for h in range(HG):
                            for po, _ in HALVES:
                                sq = p23w.tile([128, H0], f32r, tag="sqt")
                                nc.scalar.activation(
                                    sq[:], src[h][:, po:po + H0], AF.Square)
                                nc.tensor.matmul(
                                    acc[0:1, po:po + H0],
                                    ones_s[:, 0:1], sq[:],
                                    start=(h == 0), stop=(h == HG - 1))
                    stq = p23v.tile([1, 1024], f32, tag="vec")
                    nc.vector.tensor_copy(stq[:], ssq[:])
                    stk = p23v.tile([1, 1024], f32, tag="vec")
                    nc.vector.tensor_copy(stk[:], ssk[:])
                    b1i = pdram.tile([2, 1024], f32)
                    b1o = pdram.tile([2, 1024], f32)
                    nc.sync.dma_start(b1i[0:1, :], stq[:])
                    nc.sync.dma_start(b1i[1:2, :], stk[:])
                    nc.gpsimd.collective_compute(
                        "AllReduce", ADD, replica_groups=rg,
                        ins=[b1i[:].opt()], outs=[b1o[:].opt()])
                    arq = p23v.tile([1, 1024], f32, tag="vec")
                    nc.sync.dma_start(arq[:], b1o[0:1, :])
                    ark = p23v.tile([1, 1024], f32, tag="vec")
                    nc.sync.dma_start(ark[:], b1o[1:2, :])
                    # rms_q also folds the 1/sqrt(HD) score scale:
                    # 1/sqrt(128*(mean+eps))
                    sqq = p23v.tile([1, 1024], f32, tag="vec")
                    nc.scalar.activation(sqq[:], arq[:], AF.Sqrt,
                                         scale=128.0 / (2 * DIM),
                                         bias=cvec_s[0:1, 0:1])
                    sqk = p23v.tile([1, 1024], f32, tag="vec")
                    nc.scalar.activation(sqk[:], ark[:], AF.Sqrt,
                                         scale=1.0 / (2 * DIM),
                                         bias=cvec_s[0:1, 1:2])
                    rcq = p23v.tile([1, 1024], f32, tag="vec")
                    nc.vector.reciprocal(rcq[:], sqq[:])
                    rck = p23v.tile([1, 1024], f32, tag="vec")
                    nc.vector.reciprocal(rck[:], sqk[:])
                    rbq = p23.tile([128, 1024], f32, tag="rbq")
                    nc.gpsimd.partition_broadcast(rbq[:], rcq[:])
                    rbk = p23.tile([128, 1024], f32, tag="rbk")
                    nc.gpsimd.partition_broadcast(rbk[:], rck[:])

                    cosq_s, cosk_s = [], []
                    for h in range(HG):
                        t = p23.tile([128, 1024], f32, tag=f"cosq{h}")
                        nc.sync.dma_start(t[:], d_cosq.ap()[h])
                        cosq_s.append(t)
                        t = p23.tile([128, 1024], f32, tag=f"cosk{h}")
                        nc.sync.dma_start(t[:], d_cosk.ap()[h])
                        cosk_s.append(t)
                    sinq_s = p23.tile([128, 1024], f32, tag="sinq")
                    nc.sync.dma_start(sinq_s[:], d_sinq.ap())
                    sink_s = p23.tile([128, 1024], f32, tag="sink")
                    nc.sync.dma_start(sink_s[:], d_sink.ap())

                    for h in range(HG):
                        tmp = pswp.tile([128, 1024], f32, tag="swp")
                        for po, _ in HALVES:
                            nc.tensor.matmul(tmp[:, po:po + H0], swmq_s[h][:],
                                             q_sb[h][:, po:po + H0],
                                             start=True, stop=True)
                        u = p23w.tile([128, 1024], f32, tag="ropeu")
                        nc.vector.tensor_tensor(u[:], q_sb[h][:],
                                                cosq_s[h][:], op=MUL)
                        v = p23w.tile([128, 1024], f32, tag="ropev")
                        nc.vector.tensor_tensor(v[:], tmp[:], sinq_s[:],
                                                op=MUL)
                        w = p23w.tile([128, 1024], f32, tag="ropew")
                        nc.vector.tensor_tensor(w[:], u[:], v[:], op=ADD)
                        nc.vector.tensor_tensor(qro[h][:], w[:], rbq[:],
                                                op=MUL)
                    for h in range(HG):
                        tmp = pswp.tile([128, 1024], f32, tag="swp")
                        for po, _ in HALVES:
                            nc.tensor.matmul(tmp[:, po:po + H0], swmk_s[h][:],
                                             k_sb[h][:, po:po + H0],
                                             start=True, stop=True)
                        for po, so in HALVES:
                            a = p23w.tile([128, H0], f32, tag="ropea")
                            nc.vector.tensor_tensor(
                                a[:], k_sb[h][:, po:po + H0],
                                cosk_s[h][:, po:po + H0], op=MUL)
                            b = p23w.tile([128, H0], f32, tag="ropeb")
                            nc.vector.tensor_tensor(
                                b[:], tmp[:, po:po + H0],
                                sink_s[:, po:po + H0], op=MUL)
                            cc = p23w.tile([128, H0], f32, tag="ropec")
                            nc.vector.tensor_tensor(cc[:], a[:], b[:], op=ADD)
                            nc.vector.tensor_tensor(
                                kro[h][:, so:so + H0], cc[:],
                                rbk[:, po:po + H0], op=MUL)

            # ---- phase 4: attention -----------------------------------
            with (
                tc.tile_pool(name="p4", bufs=1) as p4,
                tc.tile_pool(name="p4s", bufs=2) as p4s,
                tc.tile_pool(name="p4p", bufs=3) as p4p,
                tc.tile_pool(name="p4v", bufs=4) as p4v,
                tc.tile_pool(name="p4r", bufs=3) as p4r,
            ):
                wo_s = p4.tile([128, 3 * DIM], f32r, tag="wo")
                nc.sync.dma_start(wo_s[:], d_wo.ap())

                attn_psum = tc.tile_pool(name="psc", bufs=2, space="PSUM")
                psc = attn_psum.__enter__()
                acc_psum = tc.tile_pool(name="pacc", bufs=1, space="PSUM")
                pacc = acc_psum.__enter__()

                def attn_tile(oacc, rsacc, lk, lv, qro_h, m, first, last):
                    sc = psc.tile([128, 1024], f32, tag="sc")
                    for po, _ in HALVES:
                        nc.tensor.matmul(sc[0:m, po:po + H0], lk,
                                         qro_h[:, po:po + H0],
                                         start=True, stop=True)
                    pt = p4p.tile([128, 1024], f32r, tag="pt")
                    nc.scalar.activation(pt[0:m, :], sc[0:m, :], AF.Exp)
                    for po, _ in HALVES:
                        nc.tensor.matmul(oacc[:, po:po + H0], lv,
                                         pt[0:m, po:po + H0],
                                         start=first, stop=last)
                        nc.tensor.matmul(rsacc[0:1, po:po + H0],
                                         ones_s[0:m, 0:1],
                                         pt[0:m, po:po + H0],
                                         start=first, stop=last)

                o_un, rs_l = [], []
                n_kt = NKT_C + len(NEW_TILES)          # 49 k-tiles per head
                chunks = [(0, 2048), (2048, 2048), (4096, 1280)]
                for h in range(HG):
                    oacc = pacc.tile([128, 1024], f32, tag="oacc")
                    rsacc = pacc.tile([1, 1024], f32, tag="rsacc")
                    kt = 0
                    for ci, (c0, cn) in enumerate(chunks):
                        nt = cn // 128
                        kc = p4s.tile([128, 2048], f32r, tag="kc")
                        nc.sync.dma_start(kc[:, 0:cn],
                                          d_kcT.ap()[h, :, c0:c0 + cn])
                        vc = p4s.tile([128, 2048], f32r, tag="vc")
                        nc.sync.dma_start(vc[:, 0:cn],
                                          d_vc.ap()[h, ci, :, 0:cn])
                        for j in range(nt):
                            attn_tile(oacc, rsacc,
                                      kc[:, j * 128:(j + 1) * 128],
                                      vc[:, j * 128:(j + 1) * 128],
                                      qro[h], 128, kt == 0, kt == n_kt - 1)
                            kt += 1
                    for j, sz in enumerate(NEW_TILES):
                        attn_tile(oacc, rsacc,
                                  kro[h][:, j * 128:j * 128 + sz],
                                  v_sb[j][0:sz, 128 * h:128 * (h + 1)],
                                  qro[h], sz, kt == 0, kt == n_kt - 1)
                        kt += 1
                    ou = p4.tile([128, 1024], f32, tag=f"oun{h}")
                    nc.vector.tensor_copy(ou[:], oacc[:])
                    o_un.append(ou)
                    rl = p4r.tile([1, 1024], f32r, tag="rsl")
                    nc.vector.tensor_copy(rl[:], rsacc[:])
                    rs_l.append(rl)

                acc_psum.__exit__(None, None, None)
                attn_psum.__exit__(None, None, None)

                # ---- AllReduce #2: rowsums ----------------------------
                with tc.tile_pool(name="p45ps", bufs=1,
                                  space="PSUM") as p45ps:
                    b12 = p45ps.tile([12, 1024], f32, tag="b12")
                    for po, _ in HALVES:
                        for h in range(HG):
                            nc.tensor.matmul(
                                b12[:, po:po + H0],
                                oneA_s[0:1, 12 * h:12 * (h + 1)],
                                rs_l[h][0:1, po:po + H0],
                                start=(h == 0), stop=(h == HG - 1))
                    st12 = p4.tile([12, 1024], f32, tag="st12")
                    nc.vector.tensor_copy(st12[:], b12[:])
                    b2i = pdram.tile([12, 1024], f32)
                    b2o = pdram.tile([12, 1024], f32)
                    nc.sync.dma_start(b2i[:], st12[:])
                    nc.gpsimd.collective_compute(
                        "AllReduce", ADD, replica_groups=rg,
                        ins=[b2i[:].opt()], outs=[b2o[:].opt()])
                    ar2 = p4.tile([12, 1024], f32, tag="ar2")
                    nc.sync.dma_start(ar2[:], b2o[:])
                    ar2r = p4.tile([12, 1024], f32r, tag="ar2r")
                    nc.scalar.copy(ar2r[:], ar2[:])

                    od = []
                    for h in range(HG):
                        rst = p45ps.tile([1, 1024], f32, tag=f"rst{h}")
                        for po, _ in HALVES:
                            nc.tensor.matmul(rst[0:1, po:po + H0],
                                             oneB_s[:, h:h + 1],
                                             ar2r[:, po:po + H0],
                                             start=True, stop=True)
                        sub = p4v.tile([1, 1024], f32, tag="vec")
                        nc.vector.tensor_scalar_add(sub[:], rst[:], -RS_CORR)
                        rcp = p4v.tile([1, 1024], f32, tag="vec")
                        nc.vector.reciprocal(rcp[:], sub[:])
                        rb = p4s.tile([128, 1024], f32, tag="rb")
                        nc.gpsimd.partition_broadcast(rb[:], rcp[:])
                        o = p4.tile([128, 1024], f32r, tag=f"od{h}")
                        nc.vector.tensor_tensor(o[:], o_un[h][:], rb[:],
                                                op=MUL)
                        od.append(o)

                # ---- phase 5: output projection -----------------------
                with tc.tile_pool(name="p5ps", bufs=2, space="PSUM") as p5ps:
                    for d in range(12):
                        yp = p5ps.tile([128, 1024], f32, tag="yp")
                        for po, _ in HALVES:
                            for h in range(HG):
                                nc.tensor.matmul(
                                    yp[:, po:po + H0],
                                    wo_s[:, h * DIM + 128 * d:
                                         h * DIM + 128 * (d + 1)],
                                    od[h][:, po:po + H0],
                                    start=(h == 0), stop=False)
                            nc.tensor.matmul(
                                yp[:, po:po + H0],
                                bias_s[0:1, 1152 + 128 * d:
                                       1152 + 128 * (d + 1)],
                                ones_row.broadcast_to([1, H0]),
                                start=False, stop=True)
                        ysb = p4s.tile([128, 1024], f32, tag="ysb")
                        nc.vector.tensor_copy(ysb[:], yp[:])
                        nc.sync.dma_start(d_out.ap()[d], ysb[:])

    nc.compile()
    return nc


def _rope_tables(freqs, grid, start_frame):
    f, h, w = int(grid[0]), int(grid[1]), int(grid[2])
    c = HD // 2
    s0, s1 = c - 2 * (c // 3), c // 3
    out = []
    for tab in (freqs[..., 0], freqs[..., 1]):
        tf = np.broadcast_to(tab[start_frame:start_frame + f, :s0]
                             [:, None, None, :], (f, h, w, s0))
        th = np.broadcast_to(tab[:h, s0:s0 + s1][None, :, None, :],
                             (f, h, w, s1))
        tw = np.broadcast_to(tab[:w, s0 + s1:c][None, None, :, :],
                             (f, h, w, s1))
        out.append(np.concatenate([tf, th, tw], -1).reshape(f * h * w, c))
    return out  # pc, ps each [S, 64]


def _strip(a):
    # [..., 880] -> [..., 1024] with halves at 0 and 512
    out = np.zeros(a.shape[:-1] + (1024,), np.float32)
    out[..., 0:440] = a[..., 0:440]
    out[..., 512:952] = a[..., 440:880]
    return out


def _host_prep(inputs):
    x = np.asarray(inputs["x"], np.float32)
    wq, wk, wv, wo = (np.asarray(inputs[k], np.float32)
                      for k in ("wq", "wk", "wv", "wo"))
    bq, bk, bv, bo = (np.asarray(inputs[k], np.float32)
                      for k in ("bq", "bk", "bv", "bo"))
    gq = np.asarray(inputs["gq"], np.float32)
    gk = np.asarray(inputs["gk"], np.float32)
    freqs = np.asarray(inputs["freqs"], np.float32)
    k_cache = np.asarray(inputs["k_cache"], np.float32)
    v_cache = np.asarray(inputs["v_cache"], np.float32)
    grid = np.asarray(inputs["grid_sizes"]).reshape(-1)
    cur = int(inputs["current_start"])
    assert x.shape == (1, S, DIM) and cur == CACHE_USED, (x.shape, cur)
    assert int(grid[0]) * int(grid[1]) * int(grid[2]) == S

    start_frame = cur // (int(grid[1]) * int(grid[2]))
    pc_t, ps_t = _rope_tables(freqs, grid, start_frame)   # [S, 64]

    perm = np.concatenate([np.arange(0, HD, 2), np.arange(1, HD, 2)])
    cidx = np.arange(HD) % 64
    xT_a = np.ascontiguousarray(
        x[0].T.reshape(12, 128, S).transpose(1, 0, 2).reshape(128, 12 * S))

    cos_b = np.ascontiguousarray(pc_t[:, cidx].T)         # [128, S]
    sin_b = np.ascontiguousarray(ps_t[:, cidx].T)

    in_maps = []
    for c in range(NCORES):
        g, r = c // 2, c % 2
        heads = [3 * g + h for h in range(HG)]
        row_idx = np.concatenate([128 * gh + perm for gh in heads])
        vrow_idx = np.concatenate([128 * gh + np.arange(HD) for gh in heads])

        def pack12(m):
            return np.ascontiguousarray(
                m.T.reshape(12, 128, 384).transpose(1, 0, 2)
                .reshape(128, 12 * 384))

        wq_a = pack12(wq[row_idx])
        wk_a = pack12(wk[row_idx])
        if r == 1:
            wv_a = pack12(wv[vrow_idx])
            bv_l = bv[vrow_idx]
        else:
            wv_a = np.zeros((128, 12 * 384), np.float32)
            bv_l = np.zeros(384, np.float32)
        wo_a = np.ascontiguousarray(
            wo[:, vrow_idx].T.reshape(3, 128, DIM).transpose(1, 0, 2)
            .reshape(128, 3 * DIM))
        bias_a = np.zeros((1, 2688), np.float32)
        bias_a[0, 0:384] = bq[row_idx]
        bias_a[0, 384:768] = bk[row_idx]
        bias_a[0, 768:1152] = bv_l
        if c == 0:
            bias_a[0, 1152:2688] = bo

        def side_tables(gvec, zero):
            cos3 = np.zeros((3, 128, 1024), np.float32)
            swm3 = np.zeros((3, 128, 128), np.float32)
            if zero:
                return cos3, np.zeros((128, 1024), np.float32), swm3
            for h, gh in enumerate(heads):
                gp = gvec[128 * gh + perm]                    # [128]
                cos3[h] = _strip(gp[:, None] * cos_b)
                for i in range(64):
                    swm3[h, 64 + i, i] = -gp[64 + i]
                    swm3[h, i, 64 + i] = gp[i]
            return cos3, _strip(sin_b), swm3

        cosq_a, sinq_a, swmq_a = side_tables(gq, False)
        cosk_a, sink_a, swmk_a = side_tables(gk, r == 0)

        kcT_a = np.zeros((3, 128, KC), np.float32)
        vc_a = np.zeros((3, 3, 128, 2048), np.float32)
        lo, hi = (0, KC) if r == 0 else (KC, CACHE_USED)
        n = hi - lo
        chunks = [(0, 2048), (2048, 2048), (4096, 1280)]
        for h, gh in enumerate(heads):
            kcT_a[h, :, 0:n] = k_cache[0, lo:hi, gh, :][:, perm].T
            vfull = np.zeros((KC, 128), np.float32)
            vfull[0:n] = v_cache[0, lo:hi, gh, :]
            for ci, (c0, cn) in enumerate(chunks):
                nt = cn // 128
                vc_a[h, ci, :, 0:cn] = (
                    vfull[c0:c0 + cn].reshape(nt, 128, 128)
                    .transpose(1, 0, 2).reshape(128, cn))

        oneA = np.zeros((1, 36), np.float32)
        oneB = np.zeros((12, 3), np.float32)
        for h, gh in enumerate(heads):
            oneA[0, 12 * h + gh] = 1.0
            oneB[gh, h] = 1.0

        in_maps.append(dict(
            xT=xT_a, wq=wq_a, wk=wk_a, wv=wv_a, wo=wo_a, bias=bias_a,
            cosq=cosq_a, sinq=sinq_a, cosk=cosk_a, sink=sink_a,
            swmq=swmq_a, swmk=swmk_a, kcT=np.ascontiguousarray(kcT_a),
            vc=vc_a, ones=np.ones((128, 8), np.float32),
            oneA=oneA, oneB=oneB,
            cvec=np.array([[128.0 * EPS, EPS, 0, 0, 0, 0, 0, 0]],
                          np.float32)))
    return in_maps


def kernel(**inputs):
    global _prog, last_results
    if _prog is None:
        _prog = _build()
    in_maps = _host_prep(inputs)
    r = run_bass_kernel_spmd(_prog, in_maps, core_ids=list(range(NCORES)))
    last_results = r
    acc = np.zeros((12, 128, 1024), np.float64)
    for c in range(NCORES):
        acc += r.results[c]["yT"]
    yT = np.concatenate([acc[:, :, 0:440], acc[:, :, 512:952]], axis=2)
    y = yT.reshape(DIM, S).T
    return np.ascontiguousarray(y).reshape(1, S, DIM).astype(np.float32)


# revision 9
# speedup vs baseline: 1.0189x; 1.0180x over previous
"""CausalWanSelfAttention on 8 trn2 NeuronCores.

Sharding: 8 cores = 4 head-groups (3 heads each) x 2 key-ranges.
Uniform SPMD program; per-core behavior comes only from input data:
  core c -> group g = c//2 (heads 3g..3g+2), range r = c%2.
  r=0 attends cache keys [0:5376); r=1 attends cache [5376:10560) (+192
  zero-pad) plus the 880 new keys. On r=0 cores the k-side rope tables and
  V weights are zeroed, so its "new key" scores are exactly 0 -> exp = 1,
  PV contribution 0; the constant pollution (880 + 192 = 1072 per query)
  is subtracted from the AllReduced rowsum on device.
Layouts: everything s-indexed uses 1024-col "strips" (real cols 0:440 and
512:952) so each fp32 matmul output (N=440) stays inside one PSUM bank.
Matmuls run in float32r (FP22 mantissa, full PE rate). Two tiny 8-core
AllReduces: (1) q/k rms sum-of-squares, (2) per-head softmax rowsums.
Host side: pre-transposes/permutes inputs, sums the 8 row-parallel output
partials.
"""

import sys
sys.path.insert(0, "/opt/trn_rl_repo")
import numpy as np

import concourse.bacc as bacc
import concourse.mybir as mybir
from concourse import tile
from concourse.bass_utils import run_bass_kernel_spmd

f32 = mybir.dt.float32
f32r = mybir.dt.float32r
AF = mybir.ActivationFunctionType
MUL = mybir.AluOpType.mult
ADD = mybir.AluOpType.add

DIM, NH, HD, S = 1536, 12, 128, 880
CACHE_USED = 10560            # keys already in cache (current_start)
KC = 5376                     # cache keys per core (42 tiles of 128)
NKT_C = KC // 128             # 42 cache k-tiles
NCORES, HG = 8, 3             # cores, heads per group
EPS = 1e-6
H0, H1 = 440, 512             # strip: half width, second-half offset
RS_CORR = 1072.0              # 880 (r0 zeroed-new) + 192 (r1 pad) exp(0)=1
NEW_TILES = [128, 128, 128, 128, 128, 128, 112]   # 880 new keys
HALVES = ((0, 0), (H1, 440))  # (strip col offset, compact col offset)

_prog = None
last_results = None


def _build():
    nc = bacc.Bacc(None, target_bir_lowering=False, debug=False,
                   num_devices=NCORES)

    d_xT = nc.dram_tensor("xT", [128, 12 * S], f32r, kind="ExternalInput")
    d_wq = nc.dram_tensor("wq", [128, 12 * 384], f32r, kind="ExternalInput")
    d_wk = nc.dram_tensor("wk", [128, 12 * 384], f32r, kind="ExternalInput")
    d_wv = nc.dram_tensor("wv", [128, 12 * 384], f32r, kind="ExternalInput")
    d_wo = nc.dram_tensor("wo", [128, 3 * DIM], f32r, kind="ExternalInput")
    d_bias = nc.dram_tensor("bias", [1, 2688], f32r, kind="ExternalInput")
    d_cosq = nc.dram_tensor("cosq", [3, 128, 1024], f32, kind="ExternalInput")
    d_sinq = nc.dram_tensor("sinq", [128, 1024], f32, kind="ExternalInput")
    d_cosk = nc.dram_tensor("cosk", [3, 128, 1024], f32, kind="ExternalInput")
    d_sink = nc.dram_tensor("sink", [128, 1024], f32, kind="ExternalInput")
    d_swmq = nc.dram_tensor("swmq", [3, 128, 128], f32r, kind="ExternalInput")
    d_swmk = nc.dram_tensor("swmk", [3, 128, 128], f32r, kind="ExternalInput")
    d_kcT = nc.dram_tensor("kcT", [3, 128, KC], f32r, kind="ExternalInput")
    d_vc = nc.dram_tensor("vc", [3, 3, 128, 2048], f32r, kind="ExternalInput")
    d_ones = nc.dram_tensor("ones", [128, 8], f32r, kind="ExternalInput")
    d_oneA = nc.dram_tensor("oneA", [1, 36], f32r, kind="ExternalInput")
    d_oneB = nc.dram_tensor("oneB", [12, 3], f32r, kind="ExternalInput")
    d_cvec = nc.dram_tensor("cvec", [1, 8], f32, kind="ExternalInput")
    d_out = nc.dram_tensor("yT", [12, 128, 1024], f32, kind="ExternalOutput")

    rg = [list(range(NCORES))]

    with tile.TileContext(nc) as tc:
        with (
            tc.tile_pool(name="pres", bufs=1) as pres,
            tc.tile_pool(name="pdram", bufs=1, space="DRAM") as pdram,
        ):
            # ---- small resident loads ---------------------------------
            bias_s = pres.tile([1, 2688], f32r, tag="bias")
            nc.sync.dma_start(bias_s[:], d_bias.ap())
            ones_s = pres.tile([128, 8], f32r, tag="ones")
            nc.sync.dma_start(ones_s[:], d_ones.ap())
            oneA_s = pres.tile([1, 36], f32r, tag="oneA")
            nc.sync.dma_start(oneA_s[:], d_oneA.ap())
            oneB_s = pres.tile([12, 3], f32r, tag="oneB")
            nc.sync.dma_start(oneB_s[:], d_oneB.ap())
            cvec_s = pres.tile([1, 8], f32, tag="cvec")
            nc.sync.dma_start(cvec_s[:], d_cvec.ap())
            swmq_s, swmk_s = [], []
            for h in range(HG):
                t = pres.tile([128, 128], f32r, tag=f"swmq{h}")
                nc.sync.dma_start(t[:], d_swmq.ap()[h])
                swmq_s.append(t)
                t = pres.tile([128, 128], f32r, tag=f"swmk{h}")
                nc.sync.dma_start(t[:], d_swmk.ap()[h])
                swmk_s.append(t)
            ones_row = ones_s[0:1, 0:1]

            v_sb = [pres.tile([128, 384], f32r, tag=f"vsb{st}",
                              name=f"vsb{st}") for st in range(7)]
            qro = [pres.tile([128, 1024], f32r, tag=f"qro{h}",
                             name=f"qro{h}") for h in range(HG)]
            kro = [pres.tile([128, S], f32r, tag=f"kro{h}",
                             name=f"kro{h}") for h in range(HG)]

            with tc.tile_pool(name="pqk", bufs=1) as pqk:
                # ---- phase 1: q/k/v projections -----------------------
                with (
                    tc.tile_pool(name="p1io", bufs=1) as p1io,
                    tc.tile_pool(name="ppj", bufs=2, space="PSUM") as ppj,
                ):
                    xT = p1io.tile([128, 12 * S], f32r, tag="xT")
                    nc.sync.dma_start(xT[:], d_xT.ap())
                    w_in = {}
                    for nm, dt_ in (("wq", d_wq), ("wk", d_wk), ("wv", d_wv)):
                        w = p1io.tile([128, 12 * 384], f32r, tag=nm)
                        nc.sync.dma_start(w[:], dt_.ap())
                        w_in[nm] = w

                    def qk_proj(w_s, bias_off, out_tag):
                        outs = []
                        for h in range(HG):
                            ps = ppj.tile([128, 1024], f32, tag="proj")
                            for po, so in HALVES:
                                for t in range(12):
                                    nc.tensor.matmul(
                                        ps[:, po:po + H0],
                                        w_s[:, t * 384 + 128 * h:
                                            t * 384 + 128 * (h + 1)],
                                        xT[:, t * S + so:t * S + so + H0],
                                        start=(t == 0), stop=False)
                                nc.tensor.matmul(
                                    ps[:, po:po + H0],
                                    bias_s[0:1, bias_off + 128 * h:
                                           bias_off + 128 * (h + 1)],
                                    ones_row.broadcast_to([1, H0]),
                                    start=False, stop=True)
                            sb = pqk.tile([128, 1024], f32r,
                                          tag=f"{out_tag}{h}")
                            nc.scalar.copy(sb[:], ps[:])
                            outs.append(sb)
                        return outs

                    q_sb = qk_proj(w_in["wq"], 0, "qsb")
                    k_sb = qk_proj(w_in["wk"], 384, "ksb")

                    for st in range(7):
                        s0 = st * 128
                        sz = min(128, S - s0)
                        ps = ppj.tile([128, 384], f32, tag="vproj")
                        for t in range(12):
                            nc.tensor.matmul(
                                ps[0:sz, :],
                                xT[:, t * S + s0:t * S + s0 + sz],
                                w_in["wv"][:, t * 384:(t + 1) * 384],
                                start=(t == 0), stop=False)
                        nc.tensor.matmul(ps[0:sz, :],
                                         ones_row.broadcast_to([1, sz]),
                                         bias_s[0:1, 768:1152],
                                         start=False, stop=True)
                        nc.scalar.copy(v_sb[st][0:sz, :], ps[0:sz, :])

                # ---- phases 2-3: rms sumsq, AR#1, rope ----------------
                with (
                    tc.tile_pool(name="p23", bufs=1) as p23,
                    tc.tile_pool(name="p23v", bufs=4) as p23v,
                    tc.tile_pool(name="p23w", bufs=2) as p23w,
                    tc.tile_pool(name="p23ps", bufs=1, space="PSUM") as p23ps,
                    tc.tile_pool(name="pswp", bufs=2, space="PSUM") as pswp,
                ):
                    # cos/sin tables first (DMA early, AR-independent)
                    cosq_s, cosk_s = [], []
                    for h in range(HG):
                        t = p23.tile([128, 1024], f32, tag=f"cosq{h}")
                        nc.sync.dma_start(t[:], d_cosq.ap()[h])
                        cosq_s.append(t)
                        t = p23.tile([128, 1024], f32, tag=f"cosk{h}")
                        nc.sync.dma_start(t[:], d_cosk.ap()[h])
                        cosk_s.append(t)
                    sinq_s = p23.tile([128, 1024], f32, tag="sinq")
                    nc.sync.dma_start(sinq_s[:], d_sinq.ap())
                    sink_s = p23.tile([128, 1024], f32, tag="sink")
                    nc.sync.dma_start(sink_s[:], d_sink.ap())

                    ssq = p23ps.tile([1, 1024], f32, tag="ssq")
                    ssk = p23ps.tile([1, 1024], f32, tag="ssk")
                    for acc, src in ((ssq, q_sb), (ssk, k_sb)):
                        for h in range(HG):
                            for po, _ in HALVES:
                                sq = p23w.tile([128, H0], f32r, tag="sqt")
                                nc.scalar.activation(
                                    sq[:], src[h][:, po:po + H0], AF.Square)
                                nc.tensor.matmul(
                                    acc[0:1, po:po + H0],
                                    ones_s[:, 0:1], sq[:],
                                    start=(h == 0), stop=(h == HG - 1))
                    stq = p23v.tile([1, 1024], f32, tag="vec")
                    nc.vector.tensor_copy(stq[:], ssq[:])
                    stk = p23v.tile([1, 1024], f32, tag="vec")
                    nc.vector.tensor_copy(stk[:], ssk[:])
                    b1i = pdram.tile([2, 1024], f32)
                    b1o = pdram.tile([2, 1024], f32)
                    nc.sync.dma_start(b1i[0:1, :], stq[:])
                    nc.sync.dma_start(b1i[1:2, :], stk[:])
                    nc.gpsimd.collective_compute(
                        "AllReduce", ADD, replica_groups=rg,
                        ins=[b1i[:].opt()], outs=[b1o[:].opt()])
                    # ---- AR-independent rope work (overlaps the AR) ---
                    qw_pre, kc_pre = [], []
                    for h in range(HG):
                        tmp = pswp.tile([128, 1024], f32, tag="swp")
                        for po, _ in HALVES:
                            nc.tensor.matmul(tmp[:, po:po + H0], swmq_s[h][:],
                                             q_sb[h][:, po:po + H0],
                                             start=True, stop=True)
                        u = p23w.tile([128, 1024], f32, tag="ropeu")
                        nc.vector.tensor_tensor(u[:], q_sb[h][:],
                                                cosq_s[h][:], op=MUL)
                        v = p23w.tile([128, 1024], f32, tag="ropev")
                        nc.vector.tensor_tensor(v[:], tmp[:], sinq_s[:],
                                                op=MUL)
                        w = p23.tile([128, 1024], f32, tag=f"qwpre{h}",
                                     name=f"qwpre{h}")
                        nc.vector.tensor_tensor(w[:], u[:], v[:], op=ADD)
                        qw_pre.append(w)
                    for h in range(HG):
                        tmp = pswp.tile([128, 1024], f32, tag="swp")
                        for po, _ in HALVES:
                            nc.tensor.matmul(tmp[:, po:po + H0], swmk_s[h][:],
                                             k_sb[h][:, po:po + H0],
                                             start=True, stop=True)
                        for hi, (po, so) in enumerate(HALVES):
                            a = p23w.tile([128, H0], f32, tag="ropea")
                            nc.vector.tensor_tensor(
                                a[:], k_sb[h][:, po:po + H0],
                                cosk_s[h][:, po:po + H0], op=MUL)
                            b = p23w.tile([128, H0], f32, tag="ropeb")
                            nc.vector.tensor_tensor(
                                b[:], tmp[:, po:po + H0],
                                sink_s[:, po:po + H0], op=MUL)
                            cc = p23.tile([128, H0], f32,
                                          tag=f"kcpre{h}_{hi}",
                                          name=f"kcpre{h}_{hi}")
                            nc.vector.tensor_tensor(cc[:], a[:], b[:], op=ADD)
                            kc_pre.append(cc)

                    # ---- AR-dependent tail ----------------------------
                    arq = p23v.tile([1, 1024], f32, tag="vec")
                    nc.sync.dma_start(arq[:], b1o[0:1, :])
                    ark = p23v.tile([1, 1024], f32, tag="vec")
                    nc.sync.dma_start(ark[:], b1o[1:2, :])
                    # rms_q also folds the 1/sqrt(HD) score scale:
                    # 1/sqrt(128*(mean+eps))
                    sqq = p23v.tile([1, 1024], f32, tag="vec")
                    nc.scalar.activation(sqq[:], arq[:], AF.Sqrt,
                                         scale=128.0 / (2 * DIM),
                                         bias=cvec_s[0:1, 0:1])
                    sqk = p23v.tile([1, 1024], f32, tag="vec")
                    nc.scalar.activation(sqk[:], ark[:], AF.Sqrt,
                                         scale=1.0 / (2 * DIM),
                                         bias=cvec_s[0:1, 1:2])
                    rcq = p23v.tile([1, 1024], f32, tag="vec")
                    nc.vector.reciprocal(rcq[:], sqq[:])
                    rck = p23v.tile([1, 1024], f32, tag="vec")
                    nc.vector.reciprocal(rck[:], sqk[:])
                    rbq = p23.tile([128, 1024], f32, tag="rbq")
                    nc.gpsimd.partition_broadcast(rbq[:], rcq[:])
                    rbk = p23.tile([128, 1024], f32, tag="rbk")
                    nc.gpsimd.partition_broadcast(rbk[:], rck[:])
                    for h in range(HG):
                        nc.vector.tensor_tensor(qro[h][:], qw_pre[h][:],
                                                rbq[:], op=MUL)
                        for hi, (po, so) in enumerate(HALVES):
                            nc.vector.tensor_tensor(
                                kro[h][:, so:so + H0],
                                kc_pre[2 * h + hi][:],
                                rbk[:, po:po + H0], op=MUL)



            # ---- phase 4: attention -----------------------------------
            with (
                tc.tile_pool(name="p4", bufs=1) as p4,
                tc.tile_pool(name="p4s", bufs=2) as p4s,
                tc.tile_pool(name="p4p", bufs=3) as p4p,
                tc.tile_pool(name="p4v", bufs=4) as p4v,
                tc.tile_pool(name="p4r", bufs=3) as p4r,
            ):
                wo_s = p4.tile([128, 3 * DIM], f32r, tag="wo")
                nc.sync.dma_start(wo_s[:], d_wo.ap())

                attn_psum = tc.tile_pool(name="psc", bufs=2, space="PSUM")
                psc = attn_psum.__enter__()
                acc_psum = tc.tile_pool(name="pacc", bufs=1, space="PSUM")
                pacc = acc_psum.__enter__()

                def attn_tile(oacc, rsacc, lk, lv, qro_h, m, first, last):
                    sc = psc.tile([128, 1024], f32, tag="sc")
                    for po, _ in HALVES:
                        nc.tensor.matmul(sc[0:m, po:po + H0], lk,
                                         qro_h[:, po:po + H0],
                                         start=True, stop=True)
                    pt = p4p.tile([128, 1024], f32r, tag="pt")
                    nc.scalar.activation(pt[0:m, :], sc[0:m, :], AF.Exp)
                    for po, _ in HALVES:
                        nc.tensor.matmul(oacc[:, po:po + H0], lv,
                                         pt[0:m, po:po + H0],
                                         start=first, stop=last)
                        nc.tensor.matmul(rsacc[0:1, po:po + H0],
                                         ones_s[0:m, 0:1],
                                         pt[0:m, po:po + H0],
                                         start=first, stop=last)

                o_un, rs_l = [], []
                n_kt = NKT_C + len(NEW_TILES)          # 49 k-tiles per head
                chunks = [(0, 2048), (2048, 2048), (4096, 1280)]
                for h in range(HG):
                    oacc = pacc.tile([128, 1024], f32, tag="oacc")
                    rsacc = pacc.tile([1, 1024], f32, tag="rsacc")
                    kt = 0
                    for ci, (c0, cn) in enumerate(chunks):
                        nt = cn // 128
                        kc = p4s.tile([128, 2048], f32r, tag="kc")
                        nc.sync.dma_start(kc[:, 0:cn],
                                          d_kcT.ap()[h, :, c0:c0 + cn])
                        vc = p4s.tile([128, 2048], f32r, tag="vc")
                        nc.sync.dma_start(vc[:, 0:cn],
                                          d_vc.ap()[h, ci, :, 0:cn])
                        for j in range(nt):
                            attn_tile(oacc, rsacc,
                                      kc[:, j * 128:(j + 1) * 128],
                                      vc[:, j * 128:(j + 1) * 128],
                                      qro[h], 128, kt == 0, kt == n_kt - 1)
                            kt += 1
                    for j, sz in enumerate(NEW_TILES):
                        attn_tile(oacc, rsacc,
                                  kro[h][:, j * 128:j * 128 + sz],
                                  v_sb[j][0:sz, 128 * h:128 * (h + 1)],
                                  qro[h], sz, kt == 0, kt == n_kt - 1)
                        kt += 1
                    ou = p4.tile([128, 1024], f32, tag=f"oun{h}")
                    nc.vector.tensor_copy(ou[:], oacc[:])
                    o_un.append(ou)
                    rl = p4r.tile([1, 1024], f32r, tag="rsl")
                    nc.vector.tensor_copy(rl[:], rsacc[:])
                    rs_l.append(rl)

                acc_psum.__exit__(None, None, None)
                attn_psum.__exit__(None, None, None)

                # ---- AllReduce #2: rowsums ----------------------------
                with tc.tile_pool(name="p45ps", bufs=1,
                                  space="PSUM") as p45ps:
                    b12 = p45ps.tile([12, 1024], f32, tag="b12")
                    for po, _ in HALVES:
                        for h in range(HG):
                            nc.tensor.matmul(
                                b12[:, po:po + H0],
                                oneA_s[0:1, 12 * h:12 * (h + 1)],
                                rs_l[h][0:1, po:po + H0],
                                start=(h == 0), stop=(h == HG - 1))
                    st12 = p4.tile([12, 1024], f32, tag="st12")
                    nc.vector.tensor_copy(st12[:], b12[:])
                    b2i = pdram.tile([12, 1024], f32)
                    b2o = pdram.tile([12, 1024], f32)
                    nc.sync.dma_start(b2i[:], st12[:])
                    nc.gpsimd.collective_compute(
                        "AllReduce", ADD, replica_groups=rg,
                        ins=[b2i[:].opt()], outs=[b2o[:].opt()])
                    ar2 = p4.tile([12, 1024], f32, tag="ar2")
                    nc.sync.dma_start(ar2[:], b2o[:])
                    ar2r = p4.tile([12, 1024], f32r, tag="ar2r")
                    nc.scalar.copy(ar2r[:], ar2[:])

                    od = []
                    for h in range(HG):
                        rst = p45ps.tile([1, 1024], f32, tag=f"rst{h}")
                        for po, _ in HALVES:
                            nc.tensor.matmul(rst[0:1, po:po + H0],
                                             oneB_s[:, h:h + 1],
                                             ar2r[:, po:po + H0],
                                             start=True, stop=True)
                        sub = p4v.tile([1, 1024], f32, tag="vec")
                        nc.vector.tensor_scalar_add(sub[:], rst[:], -RS_CORR)
                        rcp = p4v.tile([1, 1024], f32, tag="vec")
                        nc.vector.reciprocal(rcp[:], sub[:])
                        rb = p4s.tile([128, 1024], f32, tag="rb")
                        nc.gpsimd.partition_broadcast(rb[:], rcp[:])
                        o = p4.tile([128, 1024], f32r, tag=f"od{h}")
                        nc.vector.tensor_tensor(o[:], o_un[h][:], rb[:],
                                                op=MUL)
                        od.append(o)

                # ---- phase 5: output projection -----------------------
                with tc.tile_pool(name="p5ps", bufs=2, space="PSUM") as p5ps:
                    for d in range(12):
                        yp = p5ps.tile([128, 1024], f32, tag="yp")
                        for po, _ in HALVES:
                            for h in range(HG):
                                nc.tensor.matmul(
                                    yp[:, po:po + H0],
                                    wo_s[:, h * DIM + 128 * d:
                                         h * DIM + 128 * (d + 1)],
                                    od[h][:, po:po + H0],
                                    start=(h == 0), stop=False)
                            nc.tensor.matmul(
                                yp[:, po:po + H0],
                                bias_s[0:1, 1152 + 128 * d:
                                       1152 + 128 * (d + 1)],
                                ones_row.broadcast_to([1, H0]),
                                start=False, stop=True)
                        ysb = p4s.tile([128, 1024], f32, tag="ysb")
                        nc.vector.tensor_copy(ysb[:], yp[:])
                        nc.sync.dma_start(d_out.ap()[d], ysb[:])

    nc.compile()
    return nc


def _rope_tables(freqs, grid, start_frame):
    f, h, w = int(grid[0]), int(grid[1]), int(grid[2])
    c = HD // 2
    s0, s1 = c - 2 * (c // 3), c // 3
    out = []
    for tab in (freqs[..., 0], freqs[..., 1]):
        tf = np.broadcast_to(tab[start_frame:start_frame + f, :s0]
                             [:, None, None, :], (f, h, w, s0))
        th = np.broadcast_to(tab[:h, s0:s0 + s1][None, :, None, :],
                             (f, h, w, s1))
        tw = np.broadcast_to(tab[:w, s0 + s1:c][None, None, :, :],
                             (f, h, w, s1))
        out.append(np.concatenate([tf, th, tw], -1).reshape(f * h * w, c))
    return out  # pc, ps each [S, 64]


def _strip(a):
    # [..., 880] -> [..., 1024] with halves at 0 and 512
    out = np.zeros(a.shape[:-1] + (1024,), np.float32)
    out[..., 0:440] = a[..., 0:440]
    out[..., 512:952] = a[..., 440:880]
    return out


def _host_prep(inputs):
    x = np.asarray(inputs["x"], np.float32)
    wq, wk, wv, wo = (np.asarray(inputs[k], np.float32)
                      for k in ("wq", "wk", "wv", "wo"))
    bq, bk, bv, bo = (np.asarray(inputs[k], np.float32)
                      for k in ("bq", "bk", "bv", "bo"))
    gq = np.asarray(inputs["gq"], np.float32)
    gk = np.asarray(inputs["gk"], np.float32)
    freqs = np.asarray(inputs["freqs"], np.float32)
    k_cache = np.asarray(inputs["k_cache"], np.float32)
    v_cache = np.asarray(inputs["v_cache"], np.float32)
    grid = np.asarray(inputs["grid_sizes"]).reshape(-1)
    cur = int(inputs["current_start"])
    assert x.shape == (1, S, DIM) and cur == CACHE_USED, (x.shape, cur)
    assert int(grid[0]) * int(grid[1]) * int(grid[2]) == S

    start_frame = cur // (int(grid[1]) * int(grid[2]))
    pc_t, ps_t = _rope_tables(freqs, grid, start_frame)   # [S, 64]

    perm = np.concatenate([np.arange(0, HD, 2), np.arange(1, HD, 2)])
    cidx = np.arange(HD) % 64
    xT_a = np.ascontiguousarray(
        x[0].T.reshape(12, 128, S).transpose(1, 0, 2).reshape(128, 12 * S))

    cos_b = np.ascontiguousarray(pc_t[:, cidx].T)         # [128, S]
    sin_b = np.ascontiguousarray(ps_t[:, cidx].T)

    in_maps = []
    for c in range(NCORES):
        g, r = c // 2, c % 2
        heads = [3 * g + h for h in range(HG)]
        row_idx = np.concatenate([128 * gh + perm for gh in heads])
        vrow_idx = np.concatenate([128 * gh + np.arange(HD) for gh in heads])

        def pack12(m):
            return np.ascontiguousarray(
                m.T.reshape(12, 128, 384).transpose(1, 0, 2)
                .reshape(128, 12 * 384))

        wq_a = pack12(wq[row_idx])
        wk_a = pack12(wk[row_idx])
        if r == 1:
            wv_a = pack12(wv[vrow_idx])
            bv_l = bv[vrow_idx]
        else:
            wv_a = np.zeros((128, 12 * 384), np.float32)
            bv_l = np.zeros(384, np.float32)
        wo_a = np.ascontiguousarray(
            wo[:, vrow_idx].T.reshape(3, 128, DIM).transpose(1, 0, 2)
            .reshape(128, 3 * DIM))
        bias_a = np.zeros((1, 2688), np.float32)
        bias_a[0, 0:384] = bq[row_idx]
        bias_a[0, 384:768] = bk[row_idx]
        bias_a[0, 768:1152] = bv_l
        if c == 0:
            bias_a[0, 1152:2688] = bo

        def side_tables(gvec, zero):
            cos3 = np.zeros((3, 128, 1024), np.float32)
            swm3 = np.zeros((3, 128, 128), np.float32)
            if zero:
                return cos3, np.zeros((128, 1024), np.float32), swm3
            for h, gh in enumerate(heads):
                gp = gvec[128 * gh + perm]                    # [128]
                cos3[h] = _strip(gp[:, None] * cos_b)
                for i in range(64):
                    swm3[h, 64 + i, i] = -gp[64 + i]
                    swm3[h, i, 64 + i] = gp[i]
            return cos3, _strip(sin_b), swm3

        cosq_a, sinq_a, swmq_a = side_tables(gq, False)
        cosk_a, sink_a, swmk_a = side_tables(gk, r == 0)

        kcT_a = np.zeros((3, 128, KC), np.float32)
        vc_a = np.zeros((3, 3, 128, 2048), np.float32)
        lo, hi = (0, KC) if r == 0 else (KC, CACHE_USED)
        n = hi - lo
        chunks = [(0, 2048), (2048, 2048), (4096, 1280)]
        for h, gh in enumerate(heads):
            kcT_a[h, :, 0:n] = k_cache[0, lo:hi, gh, :][:, perm].T
            vfull = np.zeros((KC, 128), np.float32)
            vfull[0:n] = v_cache[0, lo:hi, gh, :]
            for ci, (c0, cn) in enumerate(chunks):
                nt = cn // 128
                vc_a[h, ci, :, 0:cn] = (
                    vfull[c0:c0 + cn].reshape(nt, 128, 128)
                    .transpose(1, 0, 2).reshape(128, cn))

        oneA = np.zeros((1, 36), np.float32)
        oneB = np.zeros((12, 3), np.float32)
        for h, gh in enumerate(heads):
            oneA[0, 12 * h + gh] = 1.0
            oneB[gh, h] = 1.0

        in_maps.append(dict(
            xT=xT_a, wq=wq_a, wk=wk_a, wv=wv_a, wo=wo_a, bias=bias_a,
            cosq=cosq_a, sinq=sinq_a, cosk=cosk_a, sink=sink_a,
            swmq=swmq_a, swmk=swmk_a, kcT=np.ascontiguousarray(kcT_a),
            vc=vc_a, ones=np.ones((128, 8), np.float32),
            oneA=oneA, oneB=oneB,
            cvec=np.array([[128.0 * EPS, EPS, 0, 0, 0, 0, 0, 0]],
                          np.float32)))
    return in_maps


def kernel(**inputs):
    global _prog, last_results
    if _prog is None:
        _prog = _build()
    in_maps = _host_prep(inputs)
    r = run_bass_kernel_spmd(_prog, in_maps, core_ids=list(range(NCORES)))
    last_results = r
    acc = np.zeros((12, 128, 1024), np.float64)
    for c in range(NCORES):
        acc += r.results[c]["yT"]
    yT = np.concatenate([acc[:, :, 0:440], acc[:, :, 512:952]], axis=2)
    y = yT.reshape(DIM, S).T
    return np.ascontiguousarray(y).reshape(1, S, DIM).astype(np.float32)
